# revision 31
# baseline (speedup 1.0000x reference)
"""AdaPool2d forward kernel for Trainium2 (8 NeuronCores, data-parallel).

x: [16, 64, 224, 224] f32, beta: [112, 112] f32 (clamped to [0,1]).
K=2 pooling, stride 2 -> out [16, 64, 112, 112].

out = beta * EDSCW + (1-beta) * EM where
  EDSCW = softmax-over-taps(dice(t, avg)) . taps
  EM    = softmax-over-taps(taps) . taps         (SoftPool)

Sharding: batch across 8 cores (2 batches/core); each core's 2*64 = 128
(b,c)-planes map exactly onto the 128 SBUF partitions. The host splits
the 2x2 window taps into a packed [128, 4, 12544] bf16 array per core so
every device op is a dense contiguous [128, N] elementwise op (bf16
engages the DVE 2x mode).

Math (per window, taps t, s = sum taps, avg = s/4):
  r4    = t / avg                      (in [-inf, inf])
  dsc   = 2*t*avg/(t^2+avg^2) = 2*r4/(r4^2+1)
  e     = exp(dsc) = Exp(2 * DSC1B(r4))  [DSC1B(r) ~ r/(r^2+1), fused DVE op]
  f     = exp(t)                        (safe unstabilized: |t| <= ~7)
  EDSCW = sum(e*t)/sum(e);  EM = sum(f*t)/sum(f)
Reciprocals via the BITWISE_NOT-seed Newton-Raphson custom DVE ops.
"""

import sys
import os
import numpy as np

for _p in ("/opt/trn_rl_repo", "/root/.axon_site/_ro/trn_rl_repo"):
    if os.path.isdir(_p) and _p not in sys.path:
        sys.path.insert(0, _p)

B, C, H, W = 16, 64, 224, 224
OH, OW = 112, 112
NWIN = OH * OW          # 12544 windows per plane
NCORES = 8
BPC = B // NCORES       # batches per core
P = BPC * C             # 128 planes per core == SBUF partitions

CHUNK = 784             # free-dim elements per compute tile (16 chunks)
_CHUNKS = [(o, min(CHUNK, NWIN - o)) for o in range(0, NWIN, CHUNK)]

_COMPILED = {}


def _register_dsc_op():
    """DSC1B: out = Src0 * nr1(Src0^2 + 1)  ~=  r/(r^2+1), 1-Newton-step
    reciprocal from the BITWISE_NOT exponent-flip seed (~0.2% max rel err).
    dsc = 2*r/(r^2+1) -> apply scale=2 in the downstream Exp activation."""
    from concourse import dve_ops as dvo
    from concourse.dve_spec import (
        Spec, Src0, One, Bin, AluOp, C0, C1, lower as dve_lower,
        _has_src1, sq,
    )
    from concourse.dve_uop import DveOpSpec

    if any(op.name == "DSC1B_ANT" for op in dvo.OPS):
        return next(op for op in dvo.OPS if op.name == "DSC1B_ANT")

    _x = sq(Src0) + One
    _nx = Bin(AluOp.BITWISE_NOT, _x, _x)
    _y0 = _nx * C0
    _y1 = _y0 * (C1 - _x * _y0)
    body = _y1 * Src0

    def _ref(in0, in1, c0, c1, c2):
        x = (in0.astype(np.float32) ** 2 + 1.0).astype(np.float32)
        nx = (~x.view(np.int32)).view(np.float32)
        y0 = nx * c0
        y1 = y0 * (c1 - x * y0)
        return y1 * in0.astype(np.float32)

    spec = Spec(body=body, reference=_ref)

    # compute the uops sha for this environment's lowering versions
    name = "DSC1B_ANT"
    shas = {}
    for ver in ("v3", "v4"):
        try:
            tmp = DveOpSpec(
                name=name, opcode=0, uops=dve_lower(spec, ver=ver),
                rd1_en=_has_src1(spec),
            )
            shas[ver] = tmp.sha(ver)
        except Exception:
            pass
    op = dvo.DveOp(name, spec, False, shas)
    _install_op(dvo, op)
    return op


def _install_op(dvo, op):
    dvo.OPS.append(op)
    dvo.CUSTOM_DVE_SPECS[op.name] = op.spec
    dvo._SUB_OPCODE_FOR_NAME[op.name] = dvo._CUSTOM_DVE_ROW_BASE + len(dvo.OPS) - 1
    assert max(dvo._SUB_OPCODE_FOR_NAME.values()) < 0x20


def _register_div_op():
    """DIV1NR_ANT: out = Src0 * nr1(Src1) ~= Src0/Src1 at ~0.2% max rel err
    (BITWISE_NOT seed + one Chebyshev-tuned Newton step)."""
    from concourse import dve_ops as dvo
    from concourse.dve_spec import (
        Spec, Src0, Src1, Bin, AluOp, C0, C1, lower as dve_lower, _has_src1,
    )
    from concourse.dve_uop import DveOpSpec

    if any(op.name == "DIV1NR_ANT" for op in dvo.OPS):
        return next(op for op in dvo.OPS if op.name == "DIV1NR_ANT")

    _nx = Bin(AluOp.BITWISE_NOT, Src1, Src1)
    _y0 = _nx * C0
    _y1 = _y0 * (C1 - Src1 * _y0)
    body = _y1 * Src0

    def _ref(in0, in1, c0, c1, c2):
        x = in1.astype(np.float32)
        nx = (~x.view(np.int32)).view(np.float32)
        y0 = nx * c0
        y1 = y0 * (c1 - x * y0)
        return y1 * in0.astype(np.float32)

    spec = Spec(body=body, reference=_ref)
    name = "DIV1NR_ANT"
    shas = {}
    for ver in ("v3", "v4"):
        try:
            tmp = DveOpSpec(
                name=name, opcode=0, uops=dve_lower(spec, ver=ver),
                rd1_en=_has_src1(spec),
            )
            shas[ver] = tmp.sha(ver)
        except Exception:
            pass
    op = dvo.DveOp(name, spec, False, shas)
    _install_op(dvo, op)
    return op


def _build():
    import concourse.bacc as bacc
    import concourse.mybir as mybir
    from concourse.tile import TileContext
    from concourse.dve_ops import RECIPROCAL_APPROX_FAST, RECIP_APPROX_FAST_CONSTS

    bf16 = mybir.dt.bfloat16
    Exp = mybir.ActivationFunctionType.Exp

    dsc_op = _register_dsc_op()
    div_op = _register_div_op()
    _CH = {"s0": -0.23549792, "s1": 2.0017324}
    _RC = RECIP_APPROX_FAST_CONSTS

    nc = bacc.Bacc()
    x4 = nc.declare_dram_parameter("x4", [P, 4, NWIN], bf16, isOutput=False)
    betab = nc.declare_dram_parameter("betab", [P, NWIN], bf16, isOutput=False)
    out_d = nc.declare_dram_parameter("out", [P, NWIN], bf16, isOutput=True)

    def recip_fast(v, out, in_):
        v._custom_dve(
            RECIPROCAL_APPROX_FAST, out=out, in0=in_,
            s0=_RC["s0"], s1=_RC["s1"], imm2=_RC["imm2"],
        )

    with TileContext(nc) as tc:
        with tc.tile_pool(name="pool", bufs=2) as pool:
            bb_all = pool.tile([P, NWIN], bf16, tag="bb_all", name="bb_all", bufs=1)
            nc.sync.dma_start(out=bb_all[:, :], in_=betab[:, :])

            for (o, n) in _CHUNKS:
                sl = slice(o, o + n)

                def T(tag, bufs=2):
                    return pool.tile([P, n], bf16, tag=tag, name=tag, bufs=bufs)

                def T4(tag, bufs=2):
                    return pool.tile([P, 4, n], bf16, tag=tag, name=tag,
                                     bufs=bufs)

                x4t = T4("x4t", bufs=3)
                nc.sync.dma_start(out=x4t[:, :, :], in_=x4[:, :, sl])
                t_in = [x4t[:, i, :] for i in range(4)]

                # avg = (a+b+c+d)/4 ; invr4 = 1/avg
                s01 = T("s01")
                s23 = T("s23")
                s = T("s")
                avg = T("avg")
                nc.vector.tensor_add(s01[:, :], t_in[0], t_in[1])
                nc.vector.tensor_add(s23[:, :], t_in[2], t_in[3])
                nc.vector.tensor_add(s[:, :], s01[:, :], s23[:, :])
                # +1e-12 so bf16-cancelled zero sums stay finite (dsc -> 0)
                nc.scalar.activation(
                    avg[:, :], s[:, :], mybir.ActivationFunctionType.Copy,
                    bias=1e-12, scale=0.25,
                )
                invr4 = T("invr4")
                recip_fast(nc.vector, invr4[:, :], avg[:, :])

                # packed per-tap math: wide ops cover all 4 taps at once
                r_all = T4("r_all")
                for i in range(4):
                    nc.vector.tensor_mul(r_all[:, i, :], t_in[i], invr4[:, :])
                dsc_all = T4("dsc_all")
                nc.vector._custom_dve(
                    dsc_op, out=dsc_all[:, :, :], in0=r_all[:, :, :],
                    s0=_CH["s0"], s1=_CH["s1"],
                )
                e_all = T4("e_all")
                nc.scalar.activation(e_all[:, :, :], dsc_all[:, :, :], Exp,
                                     scale=2.0)
                f_all = T4("f_all")
                nc.scalar.activation(f_all[:, :, :], x4t[:, :, :], Exp)
                pe_all = T4("pe_all")
                nc.vector.tensor_mul(pe_all[:, :, :], e_all[:, :, :],
                                     x4t[:, :, :])
                pf_all = T4("pf_all")
                # EM products: split 1 DVE / 3 GPSIMD for engine balance
                nc.vector.tensor_mul(pf_all[:, 0, :], f_all[:, 0, :], t_in[0])
                for i in range(1, 4):
                    nc.gpsimd.tensor_mul(pf_all[:, i, :], f_all[:, i, :],
                                         t_in[i])

                def tree(eng, src, tag):
                    a = T(tag + "a")
                    b = T(tag + "b")
                    x = T(tag)
                    eng.tensor_add(a[:, :], src[:, 0, :], src[:, 1, :])
                    eng.tensor_add(b[:, :], src[:, 2, :], src[:, 3, :])
                    eng.tensor_add(x[:, :], a[:, :], b[:, :])
                    return x

                E = tree(nc.vector, e_all, "E")
                Pn = tree(nc.vector, pe_all, "Pn")
                F = tree(nc.gpsimd, f_all, "F")
                Qn = tree(nc.gpsimd, pf_all, "Qn")

                edscw = T("edscw")
                nc.vector._custom_dve(
                    div_op, out=edscw[:, :], in0=Pn[:, :], in1=E[:, :],
                    s0=_CH["s0"], s1=_CH["s1"],
                )
                em = T("em")
                nc.vector._custom_dve(
                    div_op, out=em[:, :], in0=Qn[:, :], in1=F[:, :],
                    s0=_CH["s0"], s1=_CH["s1"],
                )
                # out = em + bb*(edscw - em)
                dif = T("dif")
                nc.vector.tensor_sub(dif[:, :], edscw[:, :], em[:, :])
                bd = T("bd")
                nc.vector.tensor_mul(bd[:, :], dif[:, :], bb_all[:, sl])
                ot = T("ot", bufs=3)
                nc.vector.tensor_add(ot[:, :], em[:, :], bd[:, :])
                nc.sync.dma_start(out=out_d[:, sl], in_=ot[:, :])
    nc.finalize()
    return nc


def _get_nc():
    if "nc" not in _COMPILED:
        _COMPILED["nc"] = _build()
    return _COMPILED["nc"]


def _shard_inputs(x, beta):
    """Host-side: split taps, pack to [P, 4, NWIN] bf16, broadcast beta."""
    import ml_dtypes

    bfl = ml_dtypes.bfloat16
    x = np.ascontiguousarray(x, dtype=np.float32)
    beta = np.asarray(beta, dtype=np.float32)
    bb = np.broadcast_to(beta.reshape(1, NWIN).astype(bfl), (P, NWIN))
    bb = np.ascontiguousarray(bb)
    in_maps = []
    for core in range(NCORES):
        planes = x[core * BPC:(core + 1) * BPC].reshape(P, H, W)
        # [P, 2, oh, 2, ow] -> taps [P, 4, oh*ow]
        v = planes.reshape(P, OH, 2, OW, 2)
        x4 = np.empty((P, 4, NWIN), dtype=bfl)
        x4[:, 0, :] = v[:, :, 0, :, 0].reshape(P, NWIN)
        x4[:, 1, :] = v[:, :, 0, :, 1].reshape(P, NWIN)
        x4[:, 2, :] = v[:, :, 1, :, 0].reshape(P, NWIN)
        x4[:, 3, :] = v[:, :, 1, :, 1].reshape(P, NWIN)
        in_maps.append({"x4": x4, "betab": bb})
    return in_maps


LAST = {}


def kernel(x, beta, trace=False, trace_kwargs=None):
    from concourse.bass_utils import run_bass_kernel_spmd

    nc = _get_nc()
    in_maps = _shard_inputs(np.asarray(x), np.asarray(beta))
    res = run_bass_kernel_spmd(
        nc, in_maps, core_ids=list(range(NCORES)),
        trace=trace, **(trace_kwargs or {}),
    )
    LAST["exec_time_ns"] = getattr(res, "exec_time_ns", None)
    LAST["results"] = res
    out = np.empty((B, C, OH, OW), dtype=np.float32)
    for core in range(NCORES):
        o = np.asarray(res.results[core]["out"]).astype(np.float32)
        out[core * BPC:(core + 1) * BPC] = o.reshape(BPC, C, OH, OW)
    return out
